# revision 1
# baseline (speedup 1.0000x reference)
"""KAN layer (Chebyshev deg-8) Trainium2 kernel, 8-core data-parallel.

Math: out[b] = sum_n hw[n] * (X @ C.T)[b,n] = X[b,:] @ (C.T @ hw)
            = sum_d sum_k W[d,k] * T_k(tanh(x[b,d])),  W[d,k]=(C.T@hw)[d*9+k]

Device evaluates a product-Chebyshev basis (bounded, well-conditioned in bf16):
  e1=u, e2=u^2, e3=T2*u, e4=T2^2, e5=T4*u, e6=T4*T2, e7=T4*(T2*u), e8=T4^2
  with T2=2u^2-1, T4=2*T2^2-1 built on ACT/DVE; the d-contraction runs on the
  PE as 8 matvec streams per 128-d chunk accumulating into PSUM.
Host: transposes x to [D, B] (layout prep), folds hweights into coeffs, and
solves the 9x9 basis transform for per-d PE weights.
"""
import sys
import numpy as np

sys.path.insert(0, "/opt/trn_rl_repo")

import orjson
from contextlib import ExitStack

import concourse.bass as bass
from concourse import mybir
from concourse.tile import TileContext
from concourse.bass_utils import run_bass_kernel_spmd

F32 = mybir.dt.float32
BF16 = mybir.dt.bfloat16
AF = mybir.ActivationFunctionType
OP = mybir.AluOpType

B, D, DEG1 = 32768, 256, 9
NCORES = 8
BC = B // NCORES          # 4096 batch per core
NCH = D // 128            # 2 partition chunks of dims
NT = 8                    # streamed basis tensors
BLK = 2048                # free-dim block for pipelining
NBLK = BC // BLK

# ---- walrus workaround: split >1 sem-waits onto Drain carriers -------------
_MAXW = 1

def _split_waits(bir_json: bytes) -> bytes:
    d = orjson.loads(bir_json)
    for fn in d.get("functions", []):
        for bb in fn.get("blocks", []):
            out = []
            for ins in bb.get("instructions", []):
                si = ins.get("sync_info") or {}
                waits = si.get("on_wait") or []
                if len(waits) > _MAXW:
                    extra, keep = waits[:-_MAXW], waits[-_MAXW:]
                    for i in range(0, len(extra), _MAXW):
                        out.append({
                            "debug": ins.get("debug", 0),
                            "engine": ins["engine"], "ins": [], "outs": [],
                            "name": f"{ins['name']}_ws{i}", "opcode": "Drain",
                            "sync_info": {"on_update": [],
                                          "on_wait": extra[i:i + _MAXW]},
                        })
                    si["on_wait"] = keep
                out.append(ins)
            bb["instructions"] = out
    return orjson.dumps(d)

def _install_patch():
    import concourse.bass_utils as bu
    if getattr(bu, "_ws_patched", False):
        return
    orig = bu.compile_bir_kernel
    def patched(bir_json, tmpdir, neff_name="file.neff"):
        return orig(_split_waits(bir_json), tmpdir, neff_name)
    bu.compile_bir_kernel = patched
    bu._ws_patched = True
    try:
        import concourse.bass2jax as b2j
        if getattr(b2j, "compile_bir_kernel", None) is orig:
            b2j.compile_bir_kernel = patched
    except Exception:
        pass

# ---- basis transform (host) ------------------------------------------------
def _basis_matrix():
    A = np.zeros((9, 9))
    A[0, 0] = 1.0                    # e0 = T0
    A[1, 1] = 1.0                    # e1 = T1
    A[[0, 2], 2] = 0.5               # e2 = u^2   = (T0+T2)/2
    A[[1, 3], 3] = 0.5               # e3 = T2*T1 = (T1+T3)/2
    A[[0, 4], 4] = 0.5               # e4 = T2^2  = (T0+T4)/2
    A[[3, 5], 5] = 0.5               # e5 = T4*T1 = (T3+T5)/2
    A[[2, 6], 6] = 0.5               # e6 = T4*T2 = (T2+T6)/2
    A[[1, 3, 5, 7], 7] = 0.25        # e7 = T4*T2*T1
    A[[0, 8], 8] = 0.5               # e8 = T4^2  = (T0+T8)/2
    return A

# ---- device kernel ---------------------------------------------------------
def _build(c0: float):
    nc = bass.Bass()
    xt = nc.declare_dram_parameter("xt", [D, BC], F32, isOutput=False)
    wv = nc.declare_dram_parameter("wv", [128, NCH * NT], F32, isOutput=False)
    y = nc.declare_dram_parameter("y", [1, BC], F32, isOutput=True)

    with TileContext(nc) as tc, ExitStack() as ctx:
        cpool = ctx.enter_context(tc.tile_pool(name="const", bufs=1))
        xp = ctx.enter_context(tc.tile_pool(name="xin", bufs=3))
        fp = ctx.enter_context(tc.tile_pool(name="feat", bufs=3))
        op = ctx.enter_context(tc.tile_pool(name="outp", bufs=1))
        pp = ctx.enter_context(tc.tile_pool(name="ps", bufs=8, space="PSUM"))

        cb = cpool.tile([1, 1], F32)
        nc.vector.memset(cb[:], float(c0))
        wf = cpool.tile([128, NCH * NT], F32)
        nc.sync.dma_start(out=wf[:], in_=wv[:])
        wb = cpool.tile([128, NCH * NT], BF16)
        nc.vector.tensor_copy(wb[:], wf[:])

        res = op.tile([1, BC], F32)

        for blk in range(NBLK):
            bs = blk * BLK
            feats = [[None] * NT for _ in range(NCH)]
            for c in range(NCH):
                xtile = xp.tile([128, BLK], F32, tag="x")
                nc.sync.dma_start(out=xtile[:],
                                  in_=xt[c * 128:(c + 1) * 128, bs:bs + BLK])
                uf = fp.tile([128, BLK], F32, tag="uf")
                nc.scalar.activation(uf[:], xtile[:], AF.Tanh)
                u = fp.tile([128, BLK], BF16, tag="u")
                nc.vector.tensor_copy(u[:], uf[:])
                q2 = fp.tile([128, BLK], BF16, tag="q2")
                nc.scalar.activation(q2[:], uf[:], AF.Square)
                t2 = fp.tile([128, BLK], BF16, tag="t2")
                nc.vector.tensor_scalar(t2[:], q2[:], 2.0, -1.0, OP.mult, OP.add)
                s4 = fp.tile([128, BLK], BF16, tag="s4")
                nc.scalar.activation(s4[:], t2[:], AF.Square)
                t4 = fp.tile([128, BLK], BF16, tag="t4")
                nc.vector.tensor_scalar(t4[:], s4[:], 2.0, -1.0, OP.mult, OP.add)
                p3 = fp.tile([128, BLK], BF16, tag="p3")
                nc.vector.tensor_mul(p3[:], t2[:], u[:])
                p5 = fp.tile([128, BLK], BF16, tag="p5")
                nc.vector.tensor_mul(p5[:], t4[:], u[:])
                p6 = fp.tile([128, BLK], BF16, tag="p6")
                nc.vector.tensor_mul(p6[:], t4[:], t2[:])
                p7 = fp.tile([128, BLK], BF16, tag="p7")
                nc.vector.tensor_mul(p7[:], t4[:], p3[:])
                s8 = fp.tile([128, BLK], BF16, tag="s8")
                nc.vector.tensor_mul(s8[:], t4[:], t4[:])
                feats[c] = [u, q2, p3, s4, p5, p6, p7, s8]
            for j in range(BLK // 512):
                ps = pp.tile([1, 512], F32)
                n = 0
                for c in range(NCH):
                    for t in range(NT):
                        nc.tensor.matmul(
                            ps[:], wb[:, c * NT + t:c * NT + t + 1],
                            feats[c][t][:, j * 512:(j + 1) * 512],
                            start=(n == 0), stop=(n == 2 * NT - 1))
                        n += 1
                nc.scalar.activation(res[:, bs + j * 512:bs + (j + 1) * 512],
                                     ps[:], AF.Identity, bias=cb[:])
        nc.sync.dma_start(out=y[:], in_=res[:])
    return nc

# ---- public entry ----------------------------------------------------------
def kernel(x, coeffs, hweights, _trace=False):
    _install_patch()
    x = np.asarray(x, dtype=np.float32)
    w = (coeffs.astype(np.float64).T @ hweights.astype(np.float64))  # [2304]
    W = w.reshape(D, DEG1)                                           # [d, k]
    # quantization-compensated solve: peel leading Chebyshev components in
    # decreasing degree; each tensor's bf16 weight rounding is re-absorbed by
    # the lower-degree tensors, leftover T0 lands in the exact fp32 const.
    import ml_dtypes
    A = _basis_matrix()
    Wc = W.astype(np.float64).copy()
    lam = np.zeros((D, DEG1))
    for t in range(DEG1 - 1, 0, -1):       # e8..e1, leading cheb index == t
        lt = Wc[:, t] / A[t, t]
        ltq = lt.astype(ml_dtypes.bfloat16).astype(np.float64)
        Wc -= ltq[:, None] * A[:, t][None, :]
        lam[:, t] = ltq
    c0 = float(Wc[:, 0].sum())
    wv = np.zeros((128, NCH * NT), dtype=np.float32)
    for c in range(NCH):
        for t in range(NT):
            wv[:, c * NT + t] = lam[c * 128:(c + 1) * 128, t + 1]

    nc = _build(c0)
    xT = np.ascontiguousarray(x.T)                                   # [D, B]
    in_maps = [{"xt": np.ascontiguousarray(xT[:, i * BC:(i + 1) * BC]),
                "wv": wv} for i in range(NCORES)]
    res = run_bass_kernel_spmd(nc, in_maps, core_ids=list(range(NCORES)),
                               trace=_trace)
    out = np.concatenate([res.results[i]["y"][0] for i in range(NCORES)])
    if _trace:
        kernel._last = res
    return out.astype(np.float32)



# revision 2
# speedup vs baseline: 37402.2372x; 37402.2372x over previous
"""KAN layer (Chebyshev deg-8) Trainium2 kernel, 8-core data-parallel.

Math: out[b] = sum_n hw[n] * (X @ C.T)[b,n] = X[b,:] @ (C.T @ hw)
            = sum_d sum_k W[d,k] * T_k(tanh(x[b,d])),  W[d,k]=(C.T@hw)[d*9+k]

Device evaluates a product-Chebyshev basis (bounded, well-conditioned in bf16):
  e1=u, e2=u^2, e3=T2*u, e4=T2^2, e5=T4*u, e6=T4*T2, e7=T4*(T2*u), e8=T4^2
  with T2=2u^2-1, T4=2*T2^2-1 built on ACT/DVE; the d-contraction runs on the
  PE as 8 matvec streams per 128-d chunk accumulating into PSUM.
Host: transposes x to [D, B] (layout prep), folds hweights into coeffs, and
solves the 9x9 basis transform for per-d PE weights.
"""
import sys
import numpy as np

sys.path.insert(0, "/opt/trn_rl_repo")

import orjson
from contextlib import ExitStack

import concourse.bass as bass
from concourse import mybir
from concourse.tile import TileContext
from concourse.bass_utils import run_bass_kernel_spmd

F32 = mybir.dt.float32
BF16 = mybir.dt.bfloat16
AF = mybir.ActivationFunctionType
OP = mybir.AluOpType

B, D, DEG1 = 32768, 256, 9
NCORES = 8
BC = B // NCORES          # 4096 batch per core
NCH = D // 128            # 2 partition chunks of dims
NT = 8                    # streamed basis tensors
BLK = 2048                # free-dim block for pipelining
NBLK = BC // BLK

# ---- walrus workaround: split >1 sem-waits onto Drain carriers -------------
_MAXW = 1

def _split_waits(bir_json: bytes) -> bytes:
    d = orjson.loads(bir_json)
    for fn in d.get("functions", []):
        for bb in fn.get("blocks", []):
            out = []
            for ins in bb.get("instructions", []):
                si = ins.get("sync_info") or {}
                waits = si.get("on_wait") or []
                if len(waits) > _MAXW:
                    extra, keep = waits[:-_MAXW], waits[-_MAXW:]
                    for i in range(0, len(extra), _MAXW):
                        out.append({
                            "debug": ins.get("debug", 0),
                            "engine": ins["engine"], "ins": [], "outs": [],
                            "name": f"{ins['name']}_ws{i}", "opcode": "Drain",
                            "sync_info": {"on_update": [],
                                          "on_wait": extra[i:i + _MAXW]},
                        })
                    si["on_wait"] = keep
                out.append(ins)
            bb["instructions"] = out
    return orjson.dumps(d)

def _install_patch():
    import concourse.bass_utils as bu
    if getattr(bu, "_ws_patched", False):
        return
    orig = bu.compile_bir_kernel
    def patched(bir_json, tmpdir, neff_name="file.neff"):
        return orig(_split_waits(bir_json), tmpdir, neff_name)
    bu.compile_bir_kernel = patched
    bu._ws_patched = True
    try:
        import concourse.bass2jax as b2j
        if getattr(b2j, "compile_bir_kernel", None) is orig:
            b2j.compile_bir_kernel = patched
    except Exception:
        pass

# ---- basis transform (host) ------------------------------------------------
def _basis_matrix():
    A = np.zeros((9, 9))
    A[0, 0] = 1.0                    # e0 = T0
    A[1, 1] = 1.0                    # e1 = T1
    A[[0, 2], 2] = 0.5               # e2 = u^2   = (T0+T2)/2
    A[[1, 3], 3] = 0.5               # e3 = T2*T1 = (T1+T3)/2
    A[[0, 4], 4] = 0.5               # e4 = T2^2  = (T0+T4)/2
    A[[3, 5], 5] = 0.5               # e5 = T4*T1 = (T3+T5)/2
    A[[2, 6], 6] = 0.5               # e6 = T4*T2 = (T2+T6)/2
    A[[1, 3, 5, 7], 7] = 0.25        # e7 = T4*T2*T1
    A[[0, 8], 8] = 0.5               # e8 = T4^2  = (T0+T8)/2
    return A

# ---- device kernel ---------------------------------------------------------
def _build(c0: float):
    nc = bass.Bass()
    xt = nc.declare_dram_parameter("xt", [D, BC], F32, isOutput=False)
    wv = nc.declare_dram_parameter("wv", [128, NCH * NT], F32, isOutput=False)
    y = nc.declare_dram_parameter("y", [1, BC], F32, isOutput=True)

    with TileContext(nc) as tc, ExitStack() as ctx:
        cpool = ctx.enter_context(tc.tile_pool(name="const", bufs=1))
        xp = ctx.enter_context(tc.tile_pool(name="xin", bufs=3))
        fp = ctx.enter_context(tc.tile_pool(name="feat", bufs=3))
        op = ctx.enter_context(tc.tile_pool(name="outp", bufs=1))
        pp = ctx.enter_context(tc.tile_pool(name="ps", bufs=8, space="PSUM"))

        cb = cpool.tile([1, 1], F32)
        nc.vector.memset(cb[:], float(c0))
        wf = cpool.tile([128, NCH * NT], F32)
        nc.sync.dma_start(out=wf[:], in_=wv[:])
        wb = cpool.tile([128, NCH * NT], BF16)
        nc.vector.tensor_copy(wb[:], wf[:])

        res = op.tile([1, BC], F32)

        for blk in range(NBLK):
            bs = blk * BLK
            feats = [[None] * NT for _ in range(NCH)]
            for c in range(NCH):
                xtile = xp.tile([128, BLK], F32, tag="x")
                nc.sync.dma_start(out=xtile[:],
                                  in_=xt[c * 128:(c + 1) * 128, bs:bs + BLK])
                uf = fp.tile([128, BLK], F32, tag="uf")
                nc.scalar.activation(uf[:], xtile[:], AF.Tanh)
                u = fp.tile([128, BLK], BF16, tag="u")
                nc.vector.tensor_copy(u[:], uf[:])
                q2 = fp.tile([128, BLK], BF16, tag="q2")
                nc.scalar.activation(q2[:], uf[:], AF.Square)
                t2 = fp.tile([128, BLK], BF16, tag="t2")
                nc.vector.tensor_scalar(t2[:], q2[:], 2.0, -1.0, OP.mult, OP.add)
                s4 = fp.tile([128, BLK], BF16, tag="s4")
                nc.scalar.activation(s4[:], t2[:], AF.Square)
                t4 = fp.tile([128, BLK], BF16, tag="t4")
                nc.vector.tensor_scalar(t4[:], s4[:], 2.0, -1.0, OP.mult, OP.add)
                p3 = fp.tile([128, BLK], BF16, tag="p3")
                nc.vector.tensor_mul(p3[:], t2[:], u[:])
                p5 = fp.tile([128, BLK], BF16, tag="p5")
                nc.vector.tensor_mul(p5[:], t4[:], u[:])
                p6 = fp.tile([128, BLK], BF16, tag="p6")
                nc.vector.tensor_mul(p6[:], t4[:], t2[:])
                p7 = fp.tile([128, BLK], BF16, tag="p7")
                nc.vector.tensor_mul(p7[:], t4[:], p3[:])
                s8 = fp.tile([128, BLK], BF16, tag="s8")
                nc.vector.tensor_mul(s8[:], t4[:], t4[:])
                feats[c] = [u, q2, p3, s4, p5, p6, p7, s8]
            for j in range(BLK // 512):
                ps = pp.tile([1, 512], F32)
                n = 0
                for c in range(NCH):
                    for t in range(NT):
                        nc.tensor.matmul(
                            ps[:], wb[:, c * NT + t:c * NT + t + 1],
                            feats[c][t][:, j * 512:(j + 1) * 512],
                            start=(n == 0), stop=(n == 2 * NT - 1))
                        n += 1
                nc.scalar.activation(res[:, bs + j * 512:bs + (j + 1) * 512],
                                     ps[:], AF.Identity, bias=cb[:])
        nc.sync.dma_start(out=y[:], in_=res[:])
    return nc

# ---- public entry ----------------------------------------------------------
def kernel(x, coeffs, hweights, _trace=False):
    _install_patch()
    x = np.asarray(x, dtype=np.float32)
    w = (coeffs.astype(np.float64).T @ hweights.astype(np.float64))  # [2304]
    W = w.reshape(D, DEG1)                                           # [d, k]
    # quantization-compensated solve: peel leading Chebyshev components in
    # decreasing degree; each tensor's bf16 weight rounding is re-absorbed by
    # the lower-degree tensors, leftover T0 lands in the exact fp32 const.
    import ml_dtypes
    A = _basis_matrix()
    Wc = W.astype(np.float64).copy()
    lam = np.zeros((D, DEG1))
    for t in range(DEG1 - 1, 0, -1):       # e8..e1, leading cheb index == t
        lt = Wc[:, t] / A[t, t]
        ltq = lt.astype(ml_dtypes.bfloat16).astype(np.float64)
        Wc -= ltq[:, None] * A[:, t][None, :]
        lam[:, t] = ltq
    c0 = float(Wc[:, 0].sum())
    wv = np.zeros((128, NCH * NT), dtype=np.float32)
    for c in range(NCH):
        for t in range(NT):
            wv[:, c * NT + t] = lam[c * 128:(c + 1) * 128, t + 1]

    nc = _build(c0)
    xT = np.ascontiguousarray(x.T)                                   # [D, B]
    in_maps = [{"xt": np.ascontiguousarray(xT[:, i * BC:(i + 1) * BC]),
                "wv": wv} for i in range(NCORES)]
    tdir = None
    if _trace:
        import tempfile
        tdir = tempfile.mkdtemp(prefix="ktrace_", dir="/tmp")
    res = run_bass_kernel_spmd(nc, in_maps, core_ids=list(range(NCORES)),
                               trace=_trace, tmpdir=tdir)
    out = np.concatenate([res.results[i]["y"][0] for i in range(NCORES)])
    if _trace:
        kernel._last = res
    return out.astype(np.float32)

